# revision 2
# baseline (speedup 1.0000x reference)
"""Rolling-mean (window=60) over time axis of (2048, 3000, 8) f32, via
banded-matmul on 8 NeuronCores. Assets axis (3000) sharded 8 ways."""

import os

import numpy as np

import concourse.bacc as bacc
import concourse.mybir as mybir
import concourse.tile as tile
from concourse.bass_utils import run_bass_kernel_spmd

# Problem constants (hardcoded per harness contract)
T = 2048          # time steps
N_FULL = 3000     # assets
J = 8             # characteristics
WIN = 60          # rolling window
N_CORES = 8
S_CORE = N_FULL * J // N_CORES  # 3000 series per core, contiguous in memory
P = 128
N_TILES = T // P  # 16

# free-dim chunks of <=512 (one PSUM bank each)
CHUNKS = [(f0, min(512, S_CORE - f0)) for f0 in range(0, S_CORE, 512)]

# variant: "f32r" (fast matmul, 1/60 in weights), "f32r_w1" (fast matmul,
# exact 1.0 weights + scale in copy), "f32" (full-precision matmul)
VARIANT = os.environ.get("KERNEL_VARIANT", "f32r_w1")

MM_DT = mybir.dt.float32 if VARIANT == "f32" else mybir.dt.float32r
SCALE_IN_COPY = VARIANT == "f32r_w1"
XIN_BUFS = int(os.environ.get("KERNEL_XIN_BUFS", "4"))
YOUT_BUFS = int(os.environ.get("KERNEL_YOUT_BUFS", "3"))


def _band_weights():
    """lhsT-layout (k, m) band matrices.

    out_tile_i[m] = sum_k A[k,m] * x_i[k] + sum_k B[k,m] * x_{i-1}[k]
    A0 replaces A for tile 0 (rows < 59 get the first full window's mean).
    """
    s = np.float32(1.0) if SCALE_IN_COPY else np.float32(1.0 / WIN)
    A = np.zeros((P, P), np.float32)
    B = np.zeros((P, P), np.float32)
    A0 = np.zeros((P, P), np.float32)
    for m in range(P):
        A[max(0, m - (WIN - 1)): m + 1, m] = s
        if m < WIN - 1:
            B[m + P - (WIN - 1):, m] = s
            A0[0:WIN, m] = s
        else:
            A0[m - (WIN - 1): m + 1, m] = s
    return np.stack([A0, A, B])  # (3, 128, 128)


def _build_nc():
    nc = bacc.Bacc(
        "TRN2",
        target_bir_lowering=False,
        debug=False,
        num_devices=N_CORES,
    )
    x_dram = nc.dram_tensor("x", [T, S_CORE], MM_DT, kind="ExternalInput").ap()
    w_dram = nc.dram_tensor("w", [3, P, P], MM_DT, kind="ExternalInput").ap()
    y_dram = nc.dram_tensor(
        "y", [T, S_CORE], mybir.dt.float32, kind="ExternalOutput"
    ).ap()

    with tile.TileContext(nc) as tc:
        with (
            tc.tile_pool(name="consts", bufs=1) as cpool,
            tc.tile_pool(name="xin", bufs=XIN_BUFS) as xpool,
            tc.tile_pool(name="yout", bufs=YOUT_BUFS) as ypool,
            tc.tile_pool(name="psum", bufs=8, space="PSUM") as ppool,
        ):
            wA0 = cpool.tile([P, P], MM_DT)
            wA = cpool.tile([P, P], MM_DT)
            wB = cpool.tile([P, P], MM_DT)
            nc.sync.dma_start(out=wA0[:], in_=w_dram[0])
            nc.sync.dma_start(out=wA[:], in_=w_dram[1])
            nc.sync.dma_start(out=wB[:], in_=w_dram[2])

            x_prev = None
            for i in range(N_TILES):
                x_i = xpool.tile([P, S_CORE], MM_DT, tag="x")
                nc.sync.dma_start(out=x_i[:], in_=x_dram[P * i: P * (i + 1), :])
                y_i = ypool.tile([P, S_CORE], mybir.dt.float32, tag="y")
                for f0, fw in CHUNKS:
                    ps = ppool.tile([P, fw], mybir.dt.float32, tag="ps")
                    if i == 0:
                        nc.tensor.matmul(
                            ps[:], wA0[:], x_i[:, f0: f0 + fw],
                            start=True, stop=True,
                        )
                    else:
                        nc.tensor.matmul(
                            ps[:], wA[:], x_i[:, f0: f0 + fw],
                            start=True, stop=False,
                        )
                        nc.tensor.matmul(
                            ps[:], wB[:], x_prev[:, f0: f0 + fw],
                            start=False, stop=True,
                        )
                    if SCALE_IN_COPY:
                        nc.vector.tensor_scalar_mul(
                            y_i[:, f0: f0 + fw], ps[:], 1.0 / WIN
                        )
                    else:
                        nc.vector.tensor_copy(out=y_i[:, f0: f0 + fw], in_=ps[:])
                nc.sync.dma_start(out=y_dram[P * i: P * (i + 1), :], in_=y_i[:])
                x_prev = x_i

    nc.compile()
    return nc


_NC = None


def _get_nc():
    global _NC
    if _NC is None:
        _NC = _build_nc()
    return _NC


def kernel(data, window_size=WIN, **_unused):
    data = np.asarray(data)
    assert data.shape == (T, N_FULL, J), data.shape
    assert int(window_size) == WIN
    in_dtype = data.dtype
    data32 = np.ascontiguousarray(data, dtype=np.float32)

    w = _band_weights()
    nc = _get_nc()

    n_per = N_FULL // N_CORES  # 375 assets per core
    in_maps = []
    for c in range(N_CORES):
        sl = data32[:, c * n_per: (c + 1) * n_per, :].reshape(T, S_CORE)
        in_maps.append({"x": np.ascontiguousarray(sl), "w": w})

    res = run_bass_kernel_spmd(nc, in_maps, core_ids=list(range(N_CORES)))
    parts = [
        res.results[c]["y"].reshape(T, n_per, J) for c in range(N_CORES)
    ]
    out = np.concatenate(parts, axis=1)
    return out.astype(in_dtype, copy=False)


# revision 12
# speedup vs baseline: 1.1134x; 1.1134x over previous
"""Rolling-mean (window=60) over time axis of (2048, 3000, 8) f32, via
banded-matmul on 8 NeuronCores. Assets axis (3000) sharded 8 ways."""

import os

import numpy as np

import concourse.bacc as bacc
import concourse.mybir as mybir
import concourse.tile as tile
from concourse.bass_utils import run_bass_kernel_spmd

# Problem constants (hardcoded per harness contract)
T = 2048          # time steps
N_FULL = 3000     # assets
J = 8             # characteristics
WIN = 60          # rolling window
N_CORES = 8
S_CORE = N_FULL * J // N_CORES  # 3000 series per core, contiguous in memory
P = 128
N_TILES = T // P  # 16

# free-dim chunks of <=512 (one PSUM bank each)
CHUNKS = [(f0, min(512, S_CORE - f0)) for f0 in range(0, S_CORE, 512)]

# variant: "f32r" (fast matmul, 1/60 in weights), "f32r_w1" (fast matmul,
# exact 1.0 weights + scale in copy), "f32" (full-precision matmul),
# "bf16hl" (host splits x into bf16 hi/lo pair; exact-1.0 bf16 weights,
# 4 accumulating bf16 matmuls per tile, scale in copy; err ~4e-6)
VARIANT = os.environ.get("KERNEL_VARIANT", "bf16hl")

if VARIANT == "f32":
    MM_DT = W_DT = mybir.dt.float32
elif VARIANT == "bf16hl":
    MM_DT = W_DT = mybir.dt.bfloat16
else:
    MM_DT = W_DT = mybir.dt.float32r
SCALE_IN_COPY = VARIANT in ("f32r_w1", "bf16hl")
HILO = VARIANT == "bf16hl"
XIN_BUFS = int(os.environ.get("KERNEL_XIN_BUFS", "4"))
YOUT_BUFS = int(os.environ.get("KERNEL_YOUT_BUFS", "3"))


def _band_weights():
    """lhsT-layout (k, m) band matrices.

    out_tile_i[m] = sum_k A[k,m] * x_i[k] + sum_k B[k,m] * x_{i-1}[k]
    A0 replaces A for tile 0 (rows < 59 get the first full window's mean).
    """
    np_w = np.dtype(np.float32)
    if W_DT == mybir.dt.bfloat16:
        import ml_dtypes

        np_w = np.dtype(ml_dtypes.bfloat16)
    s = np_w.type(1.0) if SCALE_IN_COPY else np_w.type(1.0 / WIN)
    A = np.zeros((P, P), np_w)
    B = np.zeros((P, P), np_w)
    A0 = np.zeros((P, P), np_w)
    for m in range(P):
        A[max(0, m - (WIN - 1)): m + 1, m] = s
        if m < WIN - 1:
            B[m + P - (WIN - 1):, m] = s
            A0[0:WIN, m] = s
        else:
            A0[m - (WIN - 1): m + 1, m] = s
    return np.stack([A0, A, B])  # (3, 128, 128)


def _build_nc():
    nc = bacc.Bacc(
        "TRN2",
        target_bir_lowering=False,
        debug=False,
        num_devices=N_CORES,
    )
    x_shape = [T, 2, S_CORE] if HILO else [T, S_CORE]
    x_dram = nc.dram_tensor("x", x_shape, MM_DT, kind="ExternalInput").ap()
    w_dram = nc.dram_tensor("w", [3, P, P], W_DT, kind="ExternalInput").ap()
    y_dram = nc.dram_tensor(
        "y", [T, S_CORE], mybir.dt.float32, kind="ExternalOutput"
    ).ap()

    with tile.TileContext(nc) as tc:
        with (
            tc.tile_pool(name="consts", bufs=1) as cpool,
            tc.tile_pool(name="xin", bufs=XIN_BUFS) as xpool,
            tc.tile_pool(name="yout", bufs=YOUT_BUFS) as ypool,
            tc.tile_pool(name="psum", bufs=8, space="PSUM") as ppool,
        ):
            wA0 = cpool.tile([P, P], W_DT)
            wA = cpool.tile([P, P], W_DT)
            wB = cpool.tile([P, P], W_DT)
            nc.sync.dma_start(out=wA0[:], in_=w_dram[0])
            nc.sync.dma_start(out=wA[:], in_=w_dram[1])
            nc.sync.dma_start(out=wB[:], in_=w_dram[2])

            x_prev = None
            for i in range(N_TILES):
                if HILO:
                    xh = xpool.tile([P, S_CORE], MM_DT, tag="xh")
                    xl = xpool.tile([P, S_CORE], MM_DT, tag="xl")
                    nc.sync.dma_start(
                        out=xh[:], in_=x_dram[P * i: P * (i + 1), 0]
                    )
                    nc.sync.dma_start(
                        out=xl[:], in_=x_dram[P * i: P * (i + 1), 1]
                    )
                    x_i = (xh, xl)
                else:
                    x_i = xpool.tile([P, S_CORE], MM_DT, tag="x")
                    nc.sync.dma_start(
                        out=x_i[:], in_=x_dram[P * i: P * (i + 1)]
                    )
                y_i = ypool.tile([P, S_CORE], mybir.dt.float32, tag="y")
                for f0, fw in CHUNKS:
                    ps = ppool.tile([P, fw], mybir.dt.float32, tag="ps")
                    if HILO:
                        wa = wA0 if i == 0 else wA
                        nc.tensor.matmul(
                            ps[:], wa[:], x_i[0][:, f0: f0 + fw],
                            start=True, stop=False,
                        )
                        nc.tensor.matmul(
                            ps[:], wa[:], x_i[1][:, f0: f0 + fw],
                            start=False, stop=(i == 0),
                        )
                        if i > 0:
                            nc.tensor.matmul(
                                ps[:], wB[:], x_prev[0][:, f0: f0 + fw],
                                start=False, stop=False,
                            )
                            nc.tensor.matmul(
                                ps[:], wB[:], x_prev[1][:, f0: f0 + fw],
                                start=False, stop=True,
                            )
                    elif i == 0:
                        nc.tensor.matmul(
                            ps[:], wA0[:], x_i[:, f0: f0 + fw],
                            start=True, stop=True,
                        )
                    else:
                        nc.tensor.matmul(
                            ps[:], wA[:], x_i[:, f0: f0 + fw],
                            start=True, stop=False,
                        )
                        nc.tensor.matmul(
                            ps[:], wB[:], x_prev[:, f0: f0 + fw],
                            start=False, stop=True,
                        )
                    if SCALE_IN_COPY:
                        nc.vector.tensor_scalar_mul(
                            y_i[:, f0: f0 + fw], ps[:], 1.0 / WIN
                        )
                    else:
                        nc.vector.tensor_copy(out=y_i[:, f0: f0 + fw], in_=ps[:])
                nc.sync.dma_start(out=y_dram[P * i: P * (i + 1), :], in_=y_i[:])
                x_prev = x_i

    nc.compile()
    return nc


_NC = None


def _get_nc():
    global _NC
    if _NC is None:
        _NC = _build_nc()
    return _NC


def kernel(data, window_size=WIN, **_unused):
    data = np.asarray(data)
    assert data.shape == (T, N_FULL, J), data.shape
    assert int(window_size) == WIN
    in_dtype = data.dtype
    data32 = np.ascontiguousarray(data, dtype=np.float32)

    w = _band_weights()
    nc = _get_nc()

    n_per = N_FULL // N_CORES  # 375 assets per core
    if HILO:
        import ml_dtypes

        bf16 = np.dtype(ml_dtypes.bfloat16)
        flat = data32.reshape(T, N_FULL * J)
        hi = flat.astype(bf16)
        lo = (flat - hi.astype(np.float32)).astype(bf16)
        packed = np.stack([hi, lo], axis=1)  # (T, 2, N_FULL*J) bf16
    in_maps = []
    for c in range(N_CORES):
        if HILO:
            sl = packed[:, :, c * S_CORE: (c + 1) * S_CORE]
        else:
            sl = data32[:, c * n_per: (c + 1) * n_per, :].reshape(T, S_CORE)
        in_maps.append({"x": np.ascontiguousarray(sl), "w": w})

    res = run_bass_kernel_spmd(nc, in_maps, core_ids=list(range(N_CORES)))
    parts = [
        res.results[c]["y"].reshape(T, n_per, J) for c in range(N_CORES)
    ]
    out = np.concatenate(parts, axis=1)
    return out.astype(in_dtype, copy=False)


# revision 18
# speedup vs baseline: 1.1401x; 1.0239x over previous
"""Rolling-mean (window=60) over time axis of (2048, 3000, 8) f32, via
banded-matmul on 8 NeuronCores. Assets axis (3000) sharded 8 ways.

Method: out[t] = mean(x[t-59..t]) (first 59 rows backfilled with the
first window's mean). With T on SBUF partitions (tiles of 128 rows) and
series on the free axis, each output tile is two banded matmuls:
A against the current input tile + B against the previous tile, with
band matrices as the 128x128 stationary operand.

Precision: x is split host-side into bf16 hi + bf16 lo (x == hi + lo to
~17 mantissa bits; same HBM bytes as f32). Weights are exactly 1.0 in
bf16, so all products are exact and PSUM accumulates in fp32; the 1/60
scale is applied in the PSUM->SBUF copy. bf16 matmuls stream at 1
cycle/row (vs 4 for fp32), keeping TensorE well under the DMA roofline.

NOTE: matmul rhs SBUF free-element offsets must stay < 4096 — wider
packed tiles (offset 5560) crash the device (NRT_EXEC_UNIT_UNRECOVERABLE).
Hence hi and lo live in separate (128, 3000) tiles.
"""

import os

import numpy as np

import concourse.bacc as bacc
import concourse.mybir as mybir
import concourse.tile as tile
from concourse.bass_utils import run_bass_kernel_spmd

# Problem constants (hardcoded per harness contract)
T = 2048          # time steps
N_FULL = 3000     # assets
J = 8             # characteristics
WIN = 60          # rolling window
N_CORES = 8
S_CORE = N_FULL * J // N_CORES  # 3000 series per core, contiguous in memory
P = 128
N_TILES = T // P  # 16

# free-dim chunks of <=512 (one PSUM bank each)
CHUNKS = [(f0, min(512, S_CORE - f0)) for f0 in range(0, S_CORE, 512)]

# variant: "bf16hl" (default; bf16 hi/lo split, ~2.5e-6 rel err),
# "f32r" (single-pass fp32 matmul, ~2.4e-4), "f32" (4-pass fp32, ~3e-7)
VARIANT = os.environ.get("KERNEL_VARIANT", "bf16hl")

if VARIANT == "f32":
    MM_DT = W_DT = mybir.dt.float32
elif VARIANT == "bf16hl":
    MM_DT = W_DT = mybir.dt.bfloat16
else:
    MM_DT = W_DT = mybir.dt.float32r
SCALE_IN_COPY = VARIANT in ("f32r_w1", "bf16hl")
HILO = VARIANT == "bf16hl"
XIN_BUFS = int(os.environ.get("KERNEL_XIN_BUFS", "4"))
YOUT_BUFS = int(os.environ.get("KERNEL_YOUT_BUFS", "3"))
OUT_DMA_ENGINE = os.environ.get("KERNEL_OUT_DMA", "sync")


def _band_weights():
    """lhsT-layout (k, m) band matrices.

    out_tile_i[m] = sum_k A[k,m] * x_i[k] + sum_k B[k,m] * x_{i-1}[k]
    A0 replaces A for tile 0 (rows < 59 get the first full window's mean).
    """
    np_w = np.dtype(np.float32)
    if W_DT == mybir.dt.bfloat16:
        import ml_dtypes

        np_w = np.dtype(ml_dtypes.bfloat16)
    s = np_w.type(1.0) if SCALE_IN_COPY else np_w.type(1.0 / WIN)
    A = np.zeros((P, P), np_w)
    B = np.zeros((P, P), np_w)
    A0 = np.zeros((P, P), np_w)
    for m in range(P):
        A[max(0, m - (WIN - 1)): m + 1, m] = s
        if m < WIN - 1:
            B[m + P - (WIN - 1):, m] = s
            A0[0:WIN, m] = s
        else:
            A0[m - (WIN - 1): m + 1, m] = s
    return np.stack([A0, A, B])  # (3, 128, 128)


def _build_nc():
    nc = bacc.Bacc(
        "TRN2",
        target_bir_lowering=False,
        debug=False,
        num_devices=N_CORES,
    )
    x_shape = [T, 2, S_CORE] if HILO else [T, S_CORE]
    x_dram = nc.dram_tensor("x", x_shape, MM_DT, kind="ExternalInput").ap()
    w_dram = nc.dram_tensor("w", [3, P, P], W_DT, kind="ExternalInput").ap()
    y_dram = nc.dram_tensor(
        "y", [T, S_CORE], mybir.dt.float32, kind="ExternalOutput"
    ).ap()
    out_dma = nc.scalar if OUT_DMA_ENGINE == "scalar" else nc.sync

    with tile.TileContext(nc) as tc:
        with (
            tc.tile_pool(name="consts", bufs=1) as cpool,
            tc.tile_pool(name="xin", bufs=XIN_BUFS) as xpool,
            tc.tile_pool(name="yout", bufs=YOUT_BUFS) as ypool,
            tc.tile_pool(name="psum", bufs=8, space="PSUM") as ppool,
        ):
            wA0 = cpool.tile([P, P], W_DT)
            wA = cpool.tile([P, P], W_DT)
            wB = cpool.tile([P, P], W_DT)
            nc.sync.dma_start(out=wA0[:], in_=w_dram[0])
            nc.sync.dma_start(out=wA[:], in_=w_dram[1])
            nc.sync.dma_start(out=wB[:], in_=w_dram[2])

            x_prev = None
            for i in range(N_TILES):
                rows = slice(P * i, P * (i + 1))
                if HILO:
                    xh = xpool.tile([P, S_CORE], MM_DT, tag="xh")
                    xl = xpool.tile([P, S_CORE], MM_DT, tag="xl")
                    nc.sync.dma_start(out=xh[:], in_=x_dram[rows, 0])
                    nc.sync.dma_start(out=xl[:], in_=x_dram[rows, 1])
                    x_i = (xh, xl)
                else:
                    x_i = xpool.tile([P, S_CORE], MM_DT, tag="x")
                    nc.sync.dma_start(out=x_i[:], in_=x_dram[rows])
                y_i = ypool.tile([P, S_CORE], mybir.dt.float32, tag="y")
                for f0, fw in CHUNKS:
                    fs = slice(f0, f0 + fw)
                    ps = ppool.tile([P, fw], mybir.dt.float32, tag="ps")
                    if HILO:
                        wa = wA0 if i == 0 else wA
                        nc.tensor.matmul(
                            ps[:], wa[:], x_i[0][:, fs], start=True, stop=False
                        )
                        nc.tensor.matmul(
                            ps[:], wa[:], x_i[1][:, fs],
                            start=False, stop=(i == 0),
                        )
                        if i > 0:
                            nc.tensor.matmul(
                                ps[:], wB[:], x_prev[0][:, fs],
                                start=False, stop=False,
                            )
                            nc.tensor.matmul(
                                ps[:], wB[:], x_prev[1][:, fs],
                                start=False, stop=True,
                            )
                    elif i == 0:
                        nc.tensor.matmul(
                            ps[:], wA0[:], x_i[:, fs], start=True, stop=True
                        )
                    else:
                        nc.tensor.matmul(
                            ps[:], wA[:], x_i[:, fs], start=True, stop=False
                        )
                        nc.tensor.matmul(
                            ps[:], wB[:], x_prev[:, fs], start=False, stop=True
                        )
                    if SCALE_IN_COPY:
                        nc.vector.tensor_scalar_mul(y_i[:, fs], ps[:], 1.0 / WIN)
                    else:
                        nc.vector.tensor_copy(out=y_i[:, fs], in_=ps[:])
                out_dma.dma_start(out=y_dram[rows, :], in_=y_i[:])
                x_prev = x_i

    nc.compile()
    return nc


_NC = None


def _get_nc():
    global _NC
    if _NC is None:
        _NC = _build_nc()
    return _NC


def kernel(data, window_size=WIN, **_unused):
    data = np.asarray(data)
    assert data.shape == (T, N_FULL, J), data.shape
    assert int(window_size) == WIN
    in_dtype = data.dtype
    data32 = np.ascontiguousarray(data, dtype=np.float32)

    w = _band_weights()
    nc = _get_nc()

    n_per = N_FULL // N_CORES  # 375 assets per core
    if HILO:
        import ml_dtypes

        bf16 = np.dtype(ml_dtypes.bfloat16)
        flat = data32.reshape(T, N_FULL * J)
        hi = flat.astype(bf16)
        lo = (flat - hi.astype(np.float32)).astype(bf16)
        packed = np.stack([hi, lo], axis=1)  # (T, 2, N_FULL*J) bf16
    in_maps = []
    for c in range(N_CORES):
        if HILO:
            sl = packed[:, :, c * S_CORE: (c + 1) * S_CORE]
        else:
            sl = data32[:, c * n_per: (c + 1) * n_per, :].reshape(T, S_CORE)
        in_maps.append({"x": np.ascontiguousarray(sl), "w": w})

    res = run_bass_kernel_spmd(nc, in_maps, core_ids=list(range(N_CORES)))
    parts = [
        res.results[c]["y"].reshape(T, n_per, J) for c in range(N_CORES)
    ]
    out = np.concatenate(parts, axis=1)
    return out.astype(in_dtype, copy=False)


# revision 19
# speedup vs baseline: 1.2449x; 1.0920x over previous
"""Rolling-mean (window=60) over time axis of (2048, 3000, 8) f32, via
banded-matmul on 8 NeuronCores. Assets axis (3000) sharded 8 ways.

Method: out[t] = mean(x[t-59..t]) (first 59 rows backfilled with the
first window's mean). With T on SBUF partitions (tiles of 128 rows) and
series on the free axis, each output tile is two banded matmuls:
A against the current input tile + B against the previous tile, with
band matrices as the 128x128 stationary operand.

Precision: x is split host-side into bf16 hi + bf16 lo (x == hi + lo to
~17 mantissa bits; same HBM bytes as f32). Weights are exactly 1.0 in
bf16, so all products are exact and PSUM accumulates in fp32; the 1/60
scale is applied in the PSUM->SBUF copy. bf16 matmuls stream at 1
cycle/row (vs 4 for fp32), keeping TensorE well under the DMA roofline.

NOTE: matmul rhs SBUF free-element offsets must stay < 4096 — wider
packed tiles (offset 5560) crash the device (NRT_EXEC_UNIT_UNRECOVERABLE).
Hence hi and lo live in separate (128, 3000) tiles.
"""

import os

import numpy as np

import concourse.bacc as bacc
import concourse.mybir as mybir
import concourse.tile as tile
from concourse.bass_utils import run_bass_kernel_spmd

# Problem constants (hardcoded per harness contract)
T = 2048          # time steps
N_FULL = 3000     # assets
J = 8             # characteristics
WIN = 60          # rolling window
N_CORES = 8
S_CORE = N_FULL * J // N_CORES  # 3000 series per core, contiguous in memory
P = 128
N_TILES = T // P  # 16

# free-dim chunks of <=512 (one PSUM bank each)
CHUNKS = [(f0, min(512, S_CORE - f0)) for f0 in range(0, S_CORE, 512)]

# variant: "bf16hl" (default; bf16 hi/lo split, ~2.5e-6 rel err, ~160us),
# "f32r" (single-pass fp32 matmul, ~2.4e-4, ~146-165us),
# "f32" (4-pass fp32 matmul, ~3e-7, ~184us)
VARIANT = os.environ.get("KERNEL_VARIANT", "bf16hl")

if VARIANT == "f32":
    MM_DT = W_DT = mybir.dt.float32
elif VARIANT == "bf16hl":
    MM_DT = W_DT = mybir.dt.bfloat16
else:
    MM_DT = W_DT = mybir.dt.float32r
SCALE_IN_COPY = VARIANT in ("f32r_w1", "bf16hl")
HILO = VARIANT == "bf16hl"
XIN_BUFS = int(os.environ.get("KERNEL_XIN_BUFS", "4"))
YOUT_BUFS = int(os.environ.get("KERNEL_YOUT_BUFS", "3"))
OUT_DMA_ENGINE = os.environ.get("KERNEL_OUT_DMA", "sync")


def _band_weights():
    """lhsT-layout (k, m) band matrices.

    out_tile_i[m] = sum_k A[k,m] * x_i[k] + sum_k B[k,m] * x_{i-1}[k]
    A0 replaces A for tile 0 (rows < 59 get the first full window's mean).
    """
    np_w = np.dtype(np.float32)
    if W_DT == mybir.dt.bfloat16:
        import ml_dtypes

        np_w = np.dtype(ml_dtypes.bfloat16)
    s = np_w.type(1.0) if SCALE_IN_COPY else np_w.type(1.0 / WIN)
    A = np.zeros((P, P), np_w)
    B = np.zeros((P, P), np_w)
    A0 = np.zeros((P, P), np_w)
    for m in range(P):
        A[max(0, m - (WIN - 1)): m + 1, m] = s
        if m < WIN - 1:
            B[m + P - (WIN - 1):, m] = s
            A0[0:WIN, m] = s
        else:
            A0[m - (WIN - 1): m + 1, m] = s
    return np.stack([A0, A, B])  # (3, 128, 128)


def _build_nc():
    nc = bacc.Bacc(
        "TRN2",
        target_bir_lowering=False,
        debug=False,
        num_devices=N_CORES,
    )
    x_shape = [T, 2, S_CORE] if HILO else [T, S_CORE]
    x_dram = nc.dram_tensor("x", x_shape, MM_DT, kind="ExternalInput").ap()
    w_dram = nc.dram_tensor("w", [3, P, P], W_DT, kind="ExternalInput").ap()
    y_dram = nc.dram_tensor(
        "y", [T, S_CORE], mybir.dt.float32, kind="ExternalOutput"
    ).ap()
    out_dma = nc.scalar if OUT_DMA_ENGINE == "scalar" else nc.sync

    with tile.TileContext(nc) as tc:
        with (
            tc.tile_pool(name="consts", bufs=1) as cpool,
            tc.tile_pool(name="xin", bufs=XIN_BUFS) as xpool,
            tc.tile_pool(name="yout", bufs=YOUT_BUFS) as ypool,
            tc.tile_pool(name="psum", bufs=8, space="PSUM") as ppool,
        ):
            wA0 = cpool.tile([P, P], W_DT)
            wA = cpool.tile([P, P], W_DT)
            wB = cpool.tile([P, P], W_DT)
            nc.sync.dma_start(out=wA0[:], in_=w_dram[0])
            nc.sync.dma_start(out=wA[:], in_=w_dram[1])
            nc.sync.dma_start(out=wB[:], in_=w_dram[2])

            x_prev = None
            for i in range(N_TILES):
                rows = slice(P * i, P * (i + 1))
                if HILO:
                    xh = xpool.tile([P, S_CORE], MM_DT, tag="xh")
                    xl = xpool.tile([P, S_CORE], MM_DT, tag="xl")
                    nc.sync.dma_start(out=xh[:], in_=x_dram[rows, 0])
                    nc.sync.dma_start(out=xl[:], in_=x_dram[rows, 1])
                    x_i = (xh, xl)
                else:
                    x_i = xpool.tile([P, S_CORE], MM_DT, tag="x")
                    nc.sync.dma_start(out=x_i[:], in_=x_dram[rows])
                y_i = ypool.tile([P, S_CORE], mybir.dt.float32, tag="y")
                for f0, fw in CHUNKS:
                    fs = slice(f0, f0 + fw)
                    ps = ppool.tile([P, fw], mybir.dt.float32, tag="ps")
                    if HILO:
                        wa = wA0 if i == 0 else wA
                        nc.tensor.matmul(
                            ps[:], wa[:], x_i[0][:, fs], start=True, stop=False
                        )
                        nc.tensor.matmul(
                            ps[:], wa[:], x_i[1][:, fs],
                            start=False, stop=(i == 0),
                        )
                        if i > 0:
                            nc.tensor.matmul(
                                ps[:], wB[:], x_prev[0][:, fs],
                                start=False, stop=False,
                            )
                            nc.tensor.matmul(
                                ps[:], wB[:], x_prev[1][:, fs],
                                start=False, stop=True,
                            )
                    elif i == 0:
                        nc.tensor.matmul(
                            ps[:], wA0[:], x_i[:, fs], start=True, stop=True
                        )
                    else:
                        nc.tensor.matmul(
                            ps[:], wA[:], x_i[:, fs], start=True, stop=False
                        )
                        nc.tensor.matmul(
                            ps[:], wB[:], x_prev[:, fs], start=False, stop=True
                        )
                    if SCALE_IN_COPY:
                        nc.vector.tensor_scalar_mul(y_i[:, fs], ps[:], 1.0 / WIN)
                    else:
                        nc.vector.tensor_copy(out=y_i[:, fs], in_=ps[:])
                out_dma.dma_start(out=y_dram[rows, :], in_=y_i[:])
                x_prev = x_i

    nc.compile()
    return nc


_NC = None


def _get_nc():
    global _NC
    if _NC is None:
        _NC = _build_nc()
    return _NC


def kernel(data, window_size=WIN, **_unused):
    data = np.asarray(data)
    assert data.shape == (T, N_FULL, J), data.shape
    assert int(window_size) == WIN
    in_dtype = data.dtype
    data32 = np.ascontiguousarray(data, dtype=np.float32)

    w = _band_weights()
    nc = _get_nc()

    n_per = N_FULL // N_CORES  # 375 assets per core
    if HILO:
        import ml_dtypes

        bf16 = np.dtype(ml_dtypes.bfloat16)
        flat = data32.reshape(T, N_FULL * J)
        hi = flat.astype(bf16)
        lo = (flat - hi.astype(np.float32)).astype(bf16)
        packed = np.stack([hi, lo], axis=1)  # (T, 2, N_FULL*J) bf16
    in_maps = []
    for c in range(N_CORES):
        if HILO:
            sl = packed[:, :, c * S_CORE: (c + 1) * S_CORE]
        else:
            sl = data32[:, c * n_per: (c + 1) * n_per, :].reshape(T, S_CORE)
        in_maps.append({"x": np.ascontiguousarray(sl), "w": w})

    res = run_bass_kernel_spmd(nc, in_maps, core_ids=list(range(N_CORES)))
    parts = [
        res.results[c]["y"].reshape(T, n_per, J) for c in range(N_CORES)
    ]
    out = np.concatenate(parts, axis=1)
    return out.astype(in_dtype, copy=False)
